# revision 1
# baseline (speedup 1.0000x reference)
"""Trainium2 Bass kernel for nn_CombineNode_7395933684091 (gnn_message_passing).

Hierarchy: 128 leaf terms (each D=1024 -> H=32), 16 internal terms
(concat of 8 children hiddens, 256 -> 32), 1 root (concat of 16
internal hiddens, 512 -> 32); every term also has a 1-dim predict head.
All matmuls followed by tanh.

Strategy: data-parallel over batch across 8 cores (Bc = 1024 rows per
core), weights replicated. On-chip layout keeps hidden features on the
PARTITION axis ("h^T layout": tiles are [features, batch]), so every
level's contraction is a natural PE matmul and the child-concat is just
stacking partition tiles. x and all weights are repacked on the host so
every DMA is contiguous per partition.

Leaf level: 4 panels x 8 groups (4 leaves) x 8 k-chunk accumulated
[128,128]x[128,512] matmuls. The per-term predict heads ride along as
extra block-diagonal columns fused into the internal-level stationary
operand (cw) and the root-level stationary operand (rw2), so they cost
no extra PE streaming.

Matmuls stream as float32r (full-rate fp32 PE mode, ~tf32 rounding;
plain float32 runs 4x slower at full precision). f32r matmuls must
write PSUM at partition offset 0, hence the one-bank-per-node layout.
"""

import numpy as np

B, D, H = 8192, 1024, 32
L, I, CPI = 128, 16, 8
NCORES = 8
BC = B // NCORES      # 1024 batch rows per core
BN = 512              # batch tile width (one PSUM bank of f32)
NBH = BC // BN        # 2 batch halves
KC = D // 128         # 8 contraction chunks for the leaf level
NPANEL = 4            # leaf panels (8 groups of 4 leaves each)
GPP = 8               # groups per panel
NOUT = L + I + 1      # 145

MM_DT = "float32r"

_CACHE = {}


def _build_nc():
    from contextlib import ExitStack

    import concourse.mybir as mybir
    import concourse.tile as tile
    from concourse import bacc

    f32 = mybir.dt.float32
    Tanh = mybir.ActivationFunctionType.Tanh
    mmdt = getattr(mybir.dt, MM_DT)

    nc = bacc.Bacc("TRN2", target_bir_lowering=False, debug=False)

    xt = nc.dram_tensor("xt", [D, BC], mmdt, kind="ExternalInput")
    lw = nc.dram_tensor("lw", [D, L * H], mmdt, kind="ExternalInput")
    lb = nc.dram_tensor("lb", [128, 32], f32, kind="ExternalInput")
    # fused internal-trans + leaf-predict stationary: per (node i, chunk j)
    # a [128, 64] block: cols 0:32 int_W chunk, cols 32+4j+c leaf Wp diag
    cw = nc.dram_tensor("cw", [128, I * 2 * 64], mmdt, kind="ExternalInput")
    intb = nc.dram_tensor("intb", [128, 4], f32, kind="ExternalInput")
    lbp8 = nc.dram_tensor("lbp8", [8, 16], f32, kind="ExternalInput")
    # fused root-trans + int-predict stationary: per panel q a [128, 48]
    # block: cols 0:32 root_W chunk, cols 32:48 int Wp diag
    rw2 = nc.dram_tensor("rw2", [128, NPANEL * 48], mmdt, kind="ExternalInput")
    intbp = nc.dram_tensor("intbp", [16, 1], f32, kind="ExternalInput")
    rootb = nc.dram_tensor("rootb", [32, 1], f32, kind="ExternalInput")
    rootwp = nc.dram_tensor("rootwp", [32, 1], mmdt, kind="ExternalInput")
    rootbp = nc.dram_tensor("rootbp", [1, 1], f32, kind="ExternalInput")
    out = nc.dram_tensor("out", [NOUT, BC], f32, kind="ExternalOutput")

    mm = nc.tensor.matmul

    with tile.TileContext(nc) as tc, ExitStack() as ctx:
        consts = ctx.enter_context(tc.tile_pool(name="consts", bufs=1))
        wpool = ctx.enter_context(tc.tile_pool(name="wpool", bufs=3))
        work = ctx.enter_context(tc.tile_pool(name="work", bufs=4))
        keep = ctx.enter_context(tc.tile_pool(name="keep", bufs=1))
        psum = ctx.enter_context(tc.tile_pool(name="psum", bufs=1, space="PSUM"))

        # --- PE pre-warm: ~4us of dummy matmuls unthrottles the HAM clock
        # gate (PE boots at 1.2 GHz; 3.4us of sustained activity -> 2.4 GHz).
        # Uses a preloaded const AP so nothing gates the first matmul.
        warm_c = nc.const_aps.tensor(0.0, (128, 64), f32)
        pwarm = psum.tile([64, 64], f32, tag="misc", bufs=1, name="pwarm")
        # 18 f32 calls = 36 split-MMs x ~107ns cold ~= 3.9us: enough to trip
        # the HAM busy window, short enough to drain before real data lands
        for _ in range(22):
            mm(pwarm[:], warm_c, warm_c, start=True, stop=True,
               skip_group_check=True)

        # --- loads, ordered so panel-0 compute overlaps the DMA preamble:
        # (xt bn0 | wp0 cols 0:512) -> lb,cw -> xt bn1 -> wp0 cols 512:1024
        xt_sb = consts.tile([128, KC * BC], mmdt, name="xt_sb")
        wp0 = wpool.tile([128, KC * 1024], mmdt, tag="wpanel", name="wp0")
        for k in range(KC):
            nc.sync.dma_start(
                xt_sb[:, k * BC:k * BC + BN], xt[k * 128:(k + 1) * 128, 0:BN]
            )
            nc.sync.dma_start(
                wp0[:, k * 1024:k * 1024 + 512], lw[k * 128:(k + 1) * 128, 0:512]
            )
        lb_sb = consts.tile([128, 32], f32, name="lb_sb")
        nc.sync.dma_start(lb_sb[:], lb[:])
        intb_sb = consts.tile([128, 4], f32, name="intb_sb")
        nc.sync.dma_start(intb_sb[:], intb[:])
        lbp8_sb = consts.tile([8, 16], f32, name="lbp8_sb")
        nc.sync.dma_start(lbp8_sb[:], lbp8[:])
        for k in range(KC):
            nc.sync.dma_start(
                wp0[:, k * 1024 + 512:(k + 1) * 1024],
                lw[k * 128:(k + 1) * 128, 512:1024],
            )
        cw_sb = consts.tile([128, I * 2 * 64], mmdt, name="cw_sb")
        nc.sync.dma_start(cw_sb[:], cw[:])
        for k in range(KC):
            nc.sync.dma_start(
                xt_sb[:, k * BC + BN:(k + 1) * BC],
                xt[k * 128:(k + 1) * 128, BN:BC],
            )
        rw2_sb = consts.tile([128, NPANEL * 48], mmdt, name="rw2_sb")
        nc.sync.dma_start(rw2_sb[:], rw2[:])
        intbp_sb = consts.tile([16, 1], f32, name="intbp_sb")
        nc.sync.dma_start(intbp_sb[:], intbp[:])
        rootb_sb = consts.tile([32, 1], f32, name="rootb_sb")
        nc.sync.dma_start(rootb_sb[:], rootb[:])
        rootwp_sb = consts.tile([32, 1], mmdt, name="rootwp_sb")
        nc.sync.dma_start(rootwp_sb[:], rootwp[:])
        rootbp_sb = consts.tile([1, 1], f32, name="rootbp_sb")
        nc.sync.dma_start(rootbp_sb[:], rootbp[:])

        intp_sb = keep.tile([16, BC], f32, name="intp_sb")
        rootp_sb = keep.tile([1, BC], f32, name="rootp_sb")

        inth = {}  # (panel, bn) -> [128, BN] tile: int nodes 4p..4p+3 h^T

        # wp1/wp2 loads emitted up front: SP issues them right after the
        # preamble instead of FIFO-blocking behind panel-0's output stores
        wps = {0: wp0}
        for q in (1, 2):
            wps[q] = wpool.tile([128, KC * 1024], mmdt, tag="wpanel", name=f"wp{q}")
            for k in range(KC):
                nc.sync.dma_start(
                    wps[q][:, k * 1024:(k + 1) * 1024],
                    lw[k * 128:(k + 1) * 128, q * 1024:(q + 1) * 1024],
                )

        # --- leaf + internal levels ----------------------------------------
        for p in range(NPANEL):
            if p in wps:
                wp = wps[p]
            else:
                wp = wpool.tile([128, KC * 1024], mmdt, tag="wpanel", name=f"wp{p}")
                for k in range(KC):
                    nc.sync.dma_start(
                        wp[:, k * 1024:(k + 1) * 1024],
                        lw[k * 128:(k + 1) * 128, p * 1024:(p + 1) * 1024],
                    )
            for bn in range(NBH):
                ith = keep.tile([128, BN], mmdt, tag=f"inth{p}{bn}", name=f"inth{p}{bn}")

                def comb_mm(il, j, lh, pcomb):
                    """Fused internal-trans + leaf-predict matmul.

                    pcomb rows 0:32 accumulate node (4p+il)'s hidden
                    pre-activation over its two child groups; rows 32:40
                    pick up the group's 4 leaf predict dots via the
                    block-diagonal columns (zeros elsewhere)."""
                    i = 4 * p + il
                    mm(
                        pcomb[:],
                        cw_sb[:, (2 * i + j) * 64:(2 * i + j + 1) * 64],
                        lh[:],
                        start=(j == 0),
                        stop=(j == 1),
                        skip_group_check=True,
                    )

                def comb_post(il, pcomb):
                    i = 4 * p + il
                    nc.scalar.activation(
                        ith[32 * il:32 * il + 32, :],
                        pcomb[0:32, :],
                        Tanh,
                        bias=intb_sb[32 * il:32 * il + 32, p:p + 1],
                    )
                    lptmp = work.tile([8, BN], f32, tag="lp", name=f"lp{i}{bn}")
                    nc.scalar.activation(
                        lptmp[:], pcomb[32:40, :], Tanh, bias=lbp8_sb[:, i:i + 1]
                    )
                    nc.gpsimd.dma_start(
                        out[8 * i:8 * i + 8, bn * BN:bn * BN + BN], lptmp[:]
                    )

                def leaf_mm(gl, k, pg):
                    mm(
                        pg[:],
                        wp[:, k * 1024 + gl * 128:k * 1024 + (gl + 1) * 128],
                        xt_sb[:, k * BC + bn * BN:k * BC + bn * BN + BN],
                        start=(k == 0),
                        stop=(k == KC - 1),
                    )

                def leaf_tanh(gl, pg):
                    lh = work.tile([128, BN], mmdt, tag="lh", name=f"lh{p}{bn}{gl}")
                    nc.scalar.activation(
                        lh[:], pg[:], Tanh, bias=lb_sb[:, GPP * p + gl:GPP * p + gl + 1]
                    )
                    return lh

                if p == 0:
                    # k-outer over a 5-group then 3-group wave: matmuls start
                    # as soon as the first xt/wp chunks land, and the first
                    # wave keeps 5 matmuls in flight per arriving chunk
                    pend = {}
                    for g0, cnt in ((0, 5), (5, 3)):
                        pgs = [
                            psum.tile([128, BN], f32, tag="pg", bufs=5,
                                      name=f"pgko{bn}{g0}{q}")
                            for q in range(cnt)
                        ]
                        for k in range(KC):
                            for q in range(cnt):
                                leaf_mm(g0 + q, k, pgs[q])
                        for q in range(cnt):
                            gl = g0 + q
                            il, j = divmod(gl, 2)
                            if j == 0:
                                pend[il] = psum.tile([64, BN], f32, tag="pcomb",
                                                     bufs=2, name=f"pcko{bn}{il}")
                            lh = leaf_tanh(gl, pgs[q])
                            comb_mm(il, j, lh, pend[il])
                            if j == 1:
                                comb_post(il, pend.pop(il))
                else:
                    for il in range(4):
                        pcomb = psum.tile([64, BN], f32, tag="pcomb", bufs=2,
                                          name=f"pc{p}{bn}{il}")
                        for j in range(2):
                            gl = 2 * il + j
                            pg = psum.tile([128, BN], f32, tag="pg", bufs=5,
                                           name=f"pg{p}{bn}{gl}")
                            for k in range(KC):
                                leaf_mm(gl, k, pg)
                            lh = leaf_tanh(gl, pg)
                            comb_mm(il, j, lh, pcomb)
                        comb_post(il, pcomb)
                inth[(p, bn)] = ith

                if p == NPANEL - 1:
                    # fused int-predict + root for this batch half, emitted
                    # here so bn=0's tail overlaps bn=1's leaf stream
                    prc = psum.tile([48, BN], f32, tag="misc", bufs=1, name=f"prc{bn}")
                    for q in range(NPANEL):
                        mm(
                            prc[:],
                            rw2_sb[:, 48 * q:48 * (q + 1)],
                            inth[(q, bn)][:],
                            start=(q == 0),
                            stop=(q == NPANEL - 1),
                            skip_group_check=True,
                        )
                    rh = work.tile([32, BN], mmdt, tag="rh", name=f"rh{bn}")
                    nc.scalar.activation(rh[:], prc[0:32, :], Tanh,
                                         bias=rootb_sb[:, 0:1])
                    nc.scalar.activation(
                        intp_sb[:, bn * BN:bn * BN + BN], prc[32:48, :], Tanh,
                        bias=intbp_sb[:, 0:1],
                    )
                    nc.sync.dma_start(
                        out[L:L + I, bn * BN:bn * BN + BN],
                        intp_sb[:, bn * BN:bn * BN + BN],
                    )
                    prp = psum.tile([1, BN], f32, tag="misc", bufs=1, name=f"prp{bn}")
                    mm(prp[:], rootwp_sb[:], rh[:], start=True, stop=True)
                    nc.scalar.activation(
                        rootp_sb[0:1, bn * BN:bn * BN + BN], prp[:], Tanh,
                        bias=rootbp_sb[:, 0:1],
                    )
                    nc.sync.dma_start(
                        out[L + I:NOUT, bn * BN:bn * BN + BN],
                        rootp_sb[0:1, bn * BN:bn * BN + BN],
                    )

    nc.compile()
    return nc


def _pack_weights(inp):
    f = np.float32
    leaf_b = np.asarray(inp["leaf_b"], f)
    int_W = np.asarray(inp["int_W"], f)
    int_b = np.asarray(inp["int_b"], f)
    root_W = np.asarray(inp["root_W"], f)
    root_b = np.asarray(inp["root_b"], f)
    leaf_Wp = np.asarray(inp["leaf_Wp"], f)
    leaf_bp = np.asarray(inp["leaf_bp"], f)
    int_Wp = np.asarray(inp["int_Wp"], f)
    int_bp = np.asarray(inp["int_bp"], f)
    root_Wp = np.asarray(inp["root_Wp"], f)
    root_bp = np.asarray(inp["root_bp"], f)

    w = {}
    w["lw"] = np.ascontiguousarray(
        np.asarray(inp["leaf_W"], f).transpose(1, 0, 2).reshape(D, L * H)
    )
    w["lb"] = np.ascontiguousarray(leaf_b.reshape(32, 128).T)

    cw = np.zeros((128, I * 2 * 64), f)
    for i in range(I):
        for j in range(2):
            base = (2 * i + j) * 64
            # int_W chunk j of node i: rows (c*32+h) = child (4j+c) hidden h
            cw[:, base:base + 32] = int_W[i, 128 * j:128 * (j + 1), :]
            for c in range(4):
                lv = 8 * i + 4 * j + c
                cw[c * 32:(c + 1) * 32, base + 32 + 4 * j + c] = leaf_Wp[lv, :, 0]
    w["cw"] = cw
    w["intb"] = np.ascontiguousarray(int_b.reshape(4, 128).T)
    w["lbp8"] = np.ascontiguousarray(leaf_bp.reshape(16, 8).T)

    rw2 = np.zeros((128, NPANEL * 48), f)
    for q in range(NPANEL):
        rw2[:, 48 * q:48 * q + 32] = root_W[128 * q:128 * (q + 1), :]
        for c in range(4):
            iv = 4 * q + c
            rw2[c * 32:(c + 1) * 32, 48 * q + 32 + 4 * q + c] = int_Wp[iv, :, 0]
    w["rw2"] = rw2
    w["intbp"] = np.ascontiguousarray(int_bp.reshape(16, 1))
    w["rootb"] = np.ascontiguousarray(root_b.reshape(32, 1))
    w["rootwp"] = np.ascontiguousarray(root_Wp.reshape(32, 1))
    w["rootbp"] = np.ascontiguousarray(root_bp.reshape(1, 1))
    return w


def kernel(**inputs):
    from concourse.bass_utils import run_bass_kernel_spmd

    nc = _CACHE.get("nc")
    if nc is None:
        nc = _CACHE["nc"] = _build_nc()

    x = np.asarray(inputs["x"], np.float32)
    w = _pack_weights(inputs)
    in_maps = []
    for c in range(NCORES):
        m = dict(w)
        m["xt"] = np.ascontiguousarray(x[c * BC:(c + 1) * BC, :].T)
        in_maps.append(m)

    res = run_bass_kernel_spmd(nc, in_maps, core_ids=list(range(NCORES)))
    _CACHE["last_res"] = res
    outs = [res.results[c]["out"] for c in range(NCORES)]
    full = np.concatenate([o[:, :, None] for o in outs], axis=1)  # [145, B, 1]
    return full.astype(np.float32)



# revision 10
# speedup vs baseline: 1.0185x; 1.0185x over previous
"""Trainium2 Bass kernel for nn_CombineNode_7395933684091 (gnn_message_passing).

Hierarchy: 128 leaf terms (each D=1024 -> H=32), 16 internal terms
(concat of 8 children hiddens, 256 -> 32), 1 root (concat of 16
internal hiddens, 512 -> 32); every term also has a 1-dim predict head.
All matmuls followed by tanh.

Strategy: data-parallel over batch across 8 cores (Bc = 1024 rows per
core), weights replicated. On-chip layout keeps hidden features on the
PARTITION axis ("h^T layout": tiles are [features, batch]), so every
level's contraction is a natural PE matmul and the child-concat is just
stacking partition tiles.

All matmul operands are bf16 (full-rate PE like f32r, but FWL halves
LDWEIGHTS and DMA bytes drop 2x vs f32). PSUM accumulation stays fp32;
biases/activations/outputs stay fp32. Inputs are host-packed into the
exact SBUF layouts so every load is a few large fully-contiguous DMAs,
split across both HWDGE queues (SP: weight panels, ACT: x + consts) so
constants never queue behind the 8MB weight stream.

Leaf level: 4 panels x 8 groups (4 leaves) x 8 k-chunk accumulated
[128,128]x[128,512] matmuls. The per-term predict heads ride along as
extra block-diagonal columns fused into the internal-level stationary
operand (cw) and the root-level stationary operand (rw2), so they cost
no extra PE streaming. Leaf predictions accumulate into a [128, Bc]
SBUF tile (rows == out rows) flushed with one DMA per (panel, half).
"""

import numpy as np

B, D, H = 8192, 1024, 32
L, I, CPI = 128, 16, 8
NCORES = 8
BC = B // NCORES      # 1024 batch rows per core
BN = 512              # batch tile width (one PSUM bank of f32)
NBH = BC // BN        # 2 batch halves
KC = D // 128         # 8 contraction chunks for the leaf level
NPANEL = 4            # leaf panels (8 groups of 4 leaves each)
GPP = 8               # groups per panel
NOUT = L + I + 1      # 145

_CACHE = {}


def _build_nc():
    from contextlib import ExitStack

    import concourse.mybir as mybir
    import concourse.tile as tile
    from concourse import bacc

    f32 = mybir.dt.float32
    bf16 = mybir.dt.bfloat16
    Tanh = mybir.ActivationFunctionType.Tanh

    nc = bacc.Bacc("TRN2", target_bir_lowering=False, debug=False)

    # xt: [128, bn*4096 + k*512 + c] = x[bn*512+c, k*128+p] (per-core slice)
    xt = nc.dram_tensor("xt", [128, NBH * KC * BN], bf16, kind="ExternalInput")
    # wp: [128, panel*8192 + k*1024 + g*128 + j]
    lw = nc.dram_tensor("lw", [128, NPANEL * KC * 1024], bf16, kind="ExternalInput")
    lb = nc.dram_tensor("lb", [128, 32], f32, kind="ExternalInput")
    # fused internal-trans + leaf-predict stationary: per (node i, chunk j)
    # a [128, 64] block: cols 0:32 int_W chunk, cols 32+4j+c leaf Wp diag
    cw = nc.dram_tensor("cw", [128, I * 2 * 64], bf16, kind="ExternalInput")
    intb = nc.dram_tensor("intb", [128, 4], f32, kind="ExternalInput")
    lbp8 = nc.dram_tensor("lbp8", [8, 16], f32, kind="ExternalInput")
    # fused root-trans + int-predict stationary: per panel q a [128, 48]
    # block: cols 0:32 root_W chunk, cols 32:48 int Wp diag
    rw2 = nc.dram_tensor("rw2", [128, NPANEL * 48], bf16, kind="ExternalInput")
    intbp = nc.dram_tensor("intbp", [16, 1], f32, kind="ExternalInput")
    rootb = nc.dram_tensor("rootb", [32, 1], f32, kind="ExternalInput")
    rootwp = nc.dram_tensor("rootwp", [32, 1], bf16, kind="ExternalInput")
    rootbp = nc.dram_tensor("rootbp", [1, 1], f32, kind="ExternalInput")
    # leaf predicts, column-packed: outl[r, (bn*16 + i)*512 + c] is leaf
    # 8i+r at batch bn*512+c (engine partition windows must be 32-aligned,
    # so the [8, BN] predict blocks go side-by-side in columns; host unpacks)
    outl = nc.dram_tensor("outl", [8, NBH * I * BN], f32, kind="ExternalOutput")
    # int predicts rows 0:16, root predict row 16
    outi = nc.dram_tensor("outi", [17, BC], f32, kind="ExternalOutput")

    mm = nc.tensor.matmul

    with tile.TileContext(nc) as tc, ExitStack() as ctx:
        consts = ctx.enter_context(tc.tile_pool(name="consts", bufs=1))
        wpool = ctx.enter_context(tc.tile_pool(name="wpool", bufs=3))
        work = ctx.enter_context(tc.tile_pool(name="work", bufs=4))
        keep = ctx.enter_context(tc.tile_pool(name="keep", bufs=1))
        psum = ctx.enter_context(tc.tile_pool(name="psum", bufs=1, space="PSUM"))

        # --- PE pre-warm: ~3.5us of dummy matmuls unthrottles the HAM clock
        # gate (PE boots at 1.2 GHz; 3.4us of sustained activity -> 2.4 GHz).
        # A gpsimd memset (done during the engine preamble) feeds the tile so
        # no DMA gates the first matmul.
        warm_c = consts.tile([128, 128], bf16, name="warm_c")
        nc.gpsimd.memset(warm_c[:], 0.0)
        pwarm = psum.tile([128, 128], f32, tag="misc", bufs=1, name="pwarm")
        for _ in range(32):
            mm(pwarm[:], warm_c[:], warm_c[:], start=True, stop=True,
               skip_group_check=True)

        # --- loads. ACT queue: x halves + small consts; SP queue: weight
        # panels. Two HWDGE queues drain in parallel, so cw/rw2 never sit
        # behind the multi-MB weight stream.
        xt_sb = consts.tile([128, NBH * KC * BN], bf16, name="xt_sb")
        nc.scalar.dma_start(xt_sb[:, 0:2048], xt[:, 0:2048])
        lb_sb = consts.tile([128, 32], f32, name="lb_sb")
        nc.scalar.dma_start(lb_sb[:], lb[:])
        intb_sb = consts.tile([128, 4], f32, name="intb_sb")
        nc.scalar.dma_start(intb_sb[:], intb[:])
        lbp8_sb = consts.tile([8, 16], f32, name="lbp8_sb")
        nc.scalar.dma_start(lbp8_sb[:], lbp8[:])
        nc.scalar.dma_start(xt_sb[:, 2048:4096], xt[:, 2048:4096])
        cw_sb = consts.tile([128, I * 2 * 64], bf16, name="cw_sb")
        nc.scalar.dma_start(cw_sb[:], cw[:])
        nc.scalar.dma_start(xt_sb[:, 4096:8192], xt[:, 4096:8192])
        rw2_sb = consts.tile([128, NPANEL * 48], bf16, name="rw2_sb")
        nc.scalar.dma_start(rw2_sb[:], rw2[:])
        intbp_sb = consts.tile([16, 1], f32, name="intbp_sb")
        nc.scalar.dma_start(intbp_sb[:], intbp[:])
        rootb_sb = consts.tile([32, 1], f32, name="rootb_sb")
        nc.scalar.dma_start(rootb_sb[:], rootb[:])
        rootwp_sb = consts.tile([32, 1], bf16, name="rootwp_sb")
        nc.scalar.dma_start(rootwp_sb[:], rootwp[:])
        rootbp_sb = consts.tile([1, 1], f32, name="rootbp_sb")
        nc.scalar.dma_start(rootbp_sb[:], rootbp[:])

        # SP queue: panel 0 in k-quarters (the wave can start on the first),
        # panels 1-2 whole; panel 3 is emitted inside the loop (its buffer
        # only frees after panel 0's last read).
        wps = {}
        wps[0] = wpool.tile([128, KC * 1024], bf16, tag="wpanel", name="wp0")
        for q in range(4):
            nc.sync.dma_start(
                wps[0][:, q * 2048:(q + 1) * 2048], lw[:, q * 2048:(q + 1) * 2048]
            )
        for p in (1, 2):
            wps[p] = wpool.tile([128, KC * 1024], bf16, tag="wpanel", name=f"wp{p}")
            nc.sync.dma_start(wps[p][:], lw[:, p * 8192:(p + 1) * 8192])

        # leaf predictions, column-packed (bn-major) to keep every
        # activation write at partition base 0
        lpp = keep.tile([8, NBH * I * BN], f32, name="lpp")
        intp_sb = keep.tile([16, BC], f32, name="intp_sb")
        rootp_sb = keep.tile([1, BC], f32, name="rootp_sb")

        inth = {}  # (panel, bn) -> [128, BN] tile: int nodes 4p..4p+3 h^T

        # --- leaf + internal levels ----------------------------------------
        for p in range(NPANEL):
            if p in wps:
                wp = wps[p]
            else:
                wp = wpool.tile([128, KC * 1024], bf16, tag="wpanel", name=f"wp{p}")
                nc.sync.dma_start(wp[:], lw[:, p * 8192:(p + 1) * 8192])
            for bn in range(NBH):
                ith = keep.tile([128, BN], bf16, tag=f"inth{p}{bn}", name=f"inth{p}{bn}")

                def comb_mm(il, j, lh, pcomb):
                    """Fused internal-trans + leaf-predict matmul.

                    pcomb rows 0:32 accumulate node (4p+il)'s hidden
                    pre-activation over its two child groups; rows 32:40
                    pick up the group's 4 leaf predict dots via the
                    block-diagonal columns (zeros elsewhere)."""
                    i = 4 * p + il
                    mm(
                        pcomb[:],
                        cw_sb[:, (2 * i + j) * 64:(2 * i + j + 1) * 64],
                        lh[:],
                        start=(j == 0),
                        stop=(j == 1),
                        skip_group_check=True,
                    )

                def comb_post(il, pcomb):
                    i = 4 * p + il
                    nc.scalar.activation(
                        ith[32 * il:32 * il + 32, :],
                        pcomb[0:32, :],
                        Tanh,
                        bias=intb_sb[32 * il:32 * il + 32, p:p + 1],
                    )
                    nc.scalar.activation(
                        lpp[0:8, (bn * I + i) * BN:(bn * I + i + 1) * BN],
                        pcomb[32:40, :],
                        Tanh,
                        bias=lbp8_sb[:, i:i + 1],
                    )

                def leaf_mm(gl, k, pg):
                    mm(
                        pg[:],
                        wp[:, k * 1024 + gl * 128:k * 1024 + (gl + 1) * 128],
                        xt_sb[:, bn * 4096 + k * BN:bn * 4096 + (k + 1) * BN],
                        start=(k == 0),
                        stop=(k == KC - 1),
                    )

                def leaf_tanh(gl, pg):
                    lh = work.tile([128, BN], bf16, tag="lh", name=f"lh{p}{bn}{gl}")
                    nc.scalar.activation(
                        lh[:], pg[:], Tanh, bias=lb_sb[:, GPP * p + gl:GPP * p + gl + 1]
                    )
                    return lh

                if p == 0:
                    # k-outer over a 5-group then 3-group wave: matmuls start
                    # as soon as the first xt/wp chunks land, and the first
                    # wave keeps 5 matmuls in flight per arriving chunk
                    pend = {}
                    for g0, cnt in ((0, 5), (5, 3)):
                        pgs = [
                            psum.tile([128, BN], f32, tag="pg", bufs=5,
                                      name=f"pgko{bn}{g0}{q}")
                            for q in range(cnt)
                        ]
                        for k in range(KC):
                            for q in range(cnt):
                                leaf_mm(g0 + q, k, pgs[q])
                        for q in range(cnt):
                            gl = g0 + q
                            il, j = divmod(gl, 2)
                            if j == 0:
                                pend[il] = psum.tile([64, BN], f32, tag="pcomb",
                                                     bufs=2, name=f"pcko{bn}{il}")
                            lh = leaf_tanh(gl, pgs[q])
                            comb_mm(il, j, lh, pend[il])
                            if j == 1:
                                comb_post(il, pend.pop(il))
                else:
                    for il in range(4):
                        pcomb = psum.tile([64, BN], f32, tag="pcomb", bufs=2,
                                          name=f"pc{p}{bn}{il}")
                        for j in range(2):
                            gl = 2 * il + j
                            pg = psum.tile([128, BN], f32, tag="pg", bufs=5,
                                           name=f"pg{p}{bn}{gl}")
                            for k in range(KC):
                                leaf_mm(gl, k, pg)
                            lh = leaf_tanh(gl, pg)
                            comb_mm(il, j, lh, pcomb)
                        comb_post(il, pcomb)
                inth[(p, bn)] = ith

                # flush this panel's 4 leaf-predict blocks for this half
                lo = (bn * I + 4 * p) * BN
                hi = (bn * I + 4 * (p + 1)) * BN
                nc.gpsimd.dma_start(outl[0:8, lo:hi], lpp[0:8, lo:hi])

                if p == NPANEL - 1:
                    # fused int-predict + root for this batch half, emitted
                    # here so bn=0's tail overlaps bn=1's leaf stream
                    prc = psum.tile([48, BN], f32, tag="misc", bufs=1, name=f"prc{bn}")
                    for q in range(NPANEL):
                        mm(
                            prc[:],
                            rw2_sb[:, 48 * q:48 * (q + 1)],
                            inth[(q, bn)][:],
                            start=(q == 0),
                            stop=(q == NPANEL - 1),
                            skip_group_check=True,
                        )
                    rh = work.tile([32, BN], bf16, tag="rh", name=f"rh{bn}")
                    nc.scalar.activation(rh[:], prc[0:32, :], Tanh,
                                         bias=rootb_sb[:, 0:1])
                    nc.scalar.activation(
                        intp_sb[:, bn * BN:bn * BN + BN], prc[32:48, :], Tanh,
                        bias=intbp_sb[:, 0:1],
                    )
                    nc.gpsimd.dma_start(
                        outi[0:16, bn * BN:bn * BN + BN],
                        intp_sb[:, bn * BN:bn * BN + BN],
                    )
                    prp = psum.tile([1, BN], f32, tag="misc", bufs=1, name=f"prp{bn}")
                    mm(prp[:], rootwp_sb[:], rh[:], start=True, stop=True)
                    nc.scalar.activation(
                        rootp_sb[0:1, bn * BN:bn * BN + BN], prp[:], Tanh,
                        bias=rootbp_sb[:, 0:1],
                    )
                    nc.gpsimd.dma_start(
                        outi[16:17, bn * BN:bn * BN + BN],
                        rootp_sb[0:1, bn * BN:bn * BN + BN],
                    )

    nc.compile()
    return nc


def _pack_weights(inp):
    import ml_dtypes

    f = np.float32
    bf = ml_dtypes.bfloat16
    leaf_b = np.asarray(inp["leaf_b"], f)
    int_W = np.asarray(inp["int_W"], f)
    int_b = np.asarray(inp["int_b"], f)
    root_W = np.asarray(inp["root_W"], f)
    root_b = np.asarray(inp["root_b"], f)
    leaf_Wp = np.asarray(inp["leaf_Wp"], f)
    leaf_bp = np.asarray(inp["leaf_bp"], f)
    int_Wp = np.asarray(inp["int_Wp"], f)
    int_bp = np.asarray(inp["int_bp"], f)
    root_Wp = np.asarray(inp["root_Wp"], f)
    root_bp = np.asarray(inp["root_bp"], f)

    w = {}
    # lwt[d, l*32+h] = leaf_W[l, d, h]; repack to
    # [p, panel*8192 + k*1024 + q] = lwt[k*128+p, panel*1024+q]
    lwt = np.asarray(inp["leaf_W"], f).transpose(1, 0, 2).reshape(D, L * H)
    w["lw"] = np.ascontiguousarray(
        lwt.reshape(KC, 128, NPANEL, 1024).transpose(1, 2, 0, 3).reshape(
            128, NPANEL * KC * 1024)
    ).astype(bf)
    w["lb"] = np.ascontiguousarray(leaf_b.reshape(32, 128).T)

    cw = np.zeros((128, I * 2 * 64), f)
    for i in range(I):
        for j in range(2):
            base = (2 * i + j) * 64
            # int_W chunk j of node i: rows (c*32+h) = child (4j+c) hidden h
            cw[:, base:base + 32] = int_W[i, 128 * j:128 * (j + 1), :]
            for c in range(4):
                lv = 8 * i + 4 * j + c
                cw[c * 32:(c + 1) * 32, base + 32 + 4 * j + c] = leaf_Wp[lv, :, 0]
    w["cw"] = cw.astype(bf)
    w["intb"] = np.ascontiguousarray(int_b.reshape(4, 128).T)
    w["lbp8"] = np.ascontiguousarray(leaf_bp.reshape(16, 8).T)

    rw2 = np.zeros((128, NPANEL * 48), f)
    for q in range(NPANEL):
        rw2[:, 48 * q:48 * q + 32] = root_W[128 * q:128 * (q + 1), :]
        for c in range(4):
            iv = 4 * q + c
            rw2[c * 32:(c + 1) * 32, 48 * q + 32 + 4 * q + c] = int_Wp[iv, :, 0]
    w["rw2"] = rw2.astype(bf)
    w["intbp"] = np.ascontiguousarray(int_bp.reshape(16, 1))
    w["rootb"] = np.ascontiguousarray(root_b.reshape(32, 1))
    w["rootwp"] = np.ascontiguousarray(root_Wp.reshape(32, 1)).astype(bf)
    w["rootbp"] = np.ascontiguousarray(root_bp.reshape(1, 1))
    return w


def kernel(**inputs):
    import ml_dtypes

    from concourse.bass_utils import run_bass_kernel_spmd

    nc = _CACHE.get("nc")
    if nc is None:
        nc = _CACHE["nc"] = _build_nc()

    x = np.asarray(inputs["x"], np.float32)
    w = _pack_weights(inputs)
    in_maps = []
    for c in range(NCORES):
        m = dict(w)
        # [p, bn*4096 + k*512 + cc] = x[c*BC + bn*512 + cc, k*128 + p]
        xc = x[c * BC:(c + 1) * BC, :].reshape(NBH, BN, KC, 128)
        m["xt"] = np.ascontiguousarray(
            xc.transpose(3, 0, 2, 1).reshape(128, NBH * KC * BN)
        ).astype(ml_dtypes.bfloat16)
        in_maps.append(m)

    res = run_bass_kernel_spmd(nc, in_maps, core_ids=list(range(NCORES)))
    _CACHE["last_res"] = res
    outs = []
    for c in range(NCORES):
        # outl[r, (bn*16 + i)*512 + cc] -> leaf 8i+r at batch bn*512+cc
        ol = res.results[c]["outl"].reshape(8, NBH, I, BN)
        leafp = ol.transpose(2, 0, 1, 3).reshape(L, BC)
        outs.append(np.concatenate([leafp, res.results[c]["outi"]], axis=0))
    full = np.concatenate([o[:, :, None] for o in outs], axis=1)  # [145, B, 1]
    return full.astype(np.float32)


# revision 11
# speedup vs baseline: 1.0714x; 1.0520x over previous
"""Trainium2 Bass kernel for nn_CombineNode_7395933684091 (gnn_message_passing).

Hierarchy: 128 leaf terms (each D=1024 -> H=32), 16 internal terms
(concat of 8 children hiddens, 256 -> 32), 1 root (concat of 16
internal hiddens, 512 -> 32); every term also has a 1-dim predict head.
All matmuls followed by tanh.

Strategy: data-parallel over batch across 8 cores (Bc = 1024 rows per
core), weights replicated. On-chip layout keeps hidden features on the
PARTITION axis ("h^T layout": tiles are [features, batch]), so every
level's contraction is a natural PE matmul and the child-concat is just
stacking partition tiles.

All matmul operands are bf16 (full-rate PE like f32r, but FWL halves
LDWEIGHTS and DMA bytes drop 2x vs f32). PSUM accumulation stays fp32;
biases/activations/outputs stay fp32. Inputs are host-packed into the
exact SBUF layouts so every load is a few large fully-contiguous DMAs,
split across both HWDGE queues (SP: weight panels, ACT: x + consts) so
constants never queue behind the 8MB weight stream. All small constants
ride in two combined tensors (one bf16, one f32) because each extra
dma_start costs ~0.6-1.2us of queue issue time.

Leaf level: 4 panels x 8 groups (4 leaves) x 8 k-chunk accumulated
[128,128]x[128,512] matmuls. The per-term predict heads ride along as
extra block-diagonal columns fused into the internal-level stationary
operand (cw) and the root-level stationary operand (rw2), so they cost
no extra PE streaming. Internal-level matmuls are emitted one leaf
group LATE (software pipelining) so the PE never waits on the scalar
engine's tanh: by the time a comb matmul issues, its lh operand is
ready. Leaf predictions land column-packed in an [8, 2*16*512] tile
(engine partition windows must be 32-aligned) and are unpacked on host.
"""

import numpy as np

B, D, H = 8192, 1024, 32
L, I, CPI = 128, 16, 8
NCORES = 8
BC = B // NCORES      # 1024 batch rows per core
BN = 512              # batch tile width (one PSUM bank of f32)
NBH = BC // BN        # 2 batch halves
KC = D // 128         # 8 contraction chunks for the leaf level
NPANEL = 4            # leaf panels (8 groups of 4 leaves each)
GPP = 8               # groups per panel
NOUT = L + I + 1      # 145

_CACHE = {}


def _build_nc():
    from contextlib import ExitStack

    import concourse.mybir as mybir
    import concourse.tile as tile
    from concourse import bacc

    f32 = mybir.dt.float32
    bf16 = mybir.dt.bfloat16
    Tanh = mybir.ActivationFunctionType.Tanh

    nc = bacc.Bacc("TRN2", target_bir_lowering=False, debug=False)

    # xt: [128, bn*4096 + k*512 + c] = x[bn*512+c, k*128+p] (per-core slice)
    xt = nc.dram_tensor("xt", [128, NBH * KC * BN], bf16, kind="ExternalInput")
    # lw: [128, panel*8192 + k*1024 + g*128 + j]
    lw = nc.dram_tensor("lw", [128, NPANEL * KC * 1024], bf16, kind="ExternalInput")
    # cwall = cw (2048) | rw2 (192) | rootwp col (1)
    cwall = nc.dram_tensor("cwall", [128, 2241], bf16, kind="ExternalInput")
    # biasall = lb (32) | intb (4) | lbp8 (16) | intbp | rootb | rootbp
    biasall = nc.dram_tensor("biasall", [128, 55], f32, kind="ExternalInput")
    # leaf predicts, column-packed: outl[r, (bn*16 + i)*512 + c] is leaf
    # 8i+r at batch bn*512+c
    outl = nc.dram_tensor("outl", [8, NBH * I * BN], f32, kind="ExternalOutput")
    # int predicts rows 0:16, root predict row 16
    outi = nc.dram_tensor("outi", [17, BC], f32, kind="ExternalOutput")

    mm = nc.tensor.matmul

    with tile.TileContext(nc) as tc, ExitStack() as ctx:
        consts = ctx.enter_context(tc.tile_pool(name="consts", bufs=1))
        wpool = ctx.enter_context(tc.tile_pool(name="wpool", bufs=3))
        work = ctx.enter_context(tc.tile_pool(name="work", bufs=4))
        keep = ctx.enter_context(tc.tile_pool(name="keep", bufs=1))
        psum = ctx.enter_context(tc.tile_pool(name="psum", bufs=1, space="PSUM"))

        # --- PE pre-warm: ~3.5us of dummy matmuls unthrottles the HAM clock
        # gate (PE boots at 1.2 GHz; 3.4us of sustained activity -> 2.4 GHz).
        # A gpsimd memset (done during the engine preamble) feeds the tile so
        # no DMA gates the first matmul.
        warm_c = consts.tile([128, 128], bf16, name="warm_c")
        nc.gpsimd.memset(warm_c[:], 0.0)
        pwarm = psum.tile([128, 128], f32, tag="pcomb", bufs=3, name="pwarm")
        for _ in range(32):
            mm(pwarm[:], warm_c[:], warm_c[:], start=True, stop=True,
               skip_group_check=True)

        # --- loads. ACT queue: x halves then consts; SP queue: weight panels.
        xt_sb = consts.tile([128, NBH * KC * BN], bf16, name="xt_sb")
        nc.scalar.dma_start(xt_sb[:, 0:2048], xt[:, 0:2048])
        nc.scalar.dma_start(xt_sb[:, 2048:4096], xt[:, 2048:4096])
        ball_sb = consts.tile([128, 55], f32, name="ball_sb")
        nc.scalar.dma_start(ball_sb[:], biasall[:])
        cwall_sb = consts.tile([128, 2241], bf16, name="cwall_sb")
        nc.scalar.dma_start(cwall_sb[:], cwall[:])
        nc.scalar.dma_start(xt_sb[:, 4096:8192], xt[:, 4096:8192])

        cw_sb = cwall_sb[:, 0:2048]
        rw2_sb = cwall_sb[:, 2048:2240]
        rootwp_sb = cwall_sb[0:32, 2240:2241]
        lb_sb = ball_sb[:, 0:32]
        intb_sb = ball_sb[:, 32:36]
        lbp8_sb = ball_sb[0:8, 36:52]
        intbp_sb = ball_sb[0:16, 52:53]
        rootb_sb = ball_sb[0:32, 53:54]
        rootbp_sb = ball_sb[0:1, 54:55]

        # SP queue: panel 0 in k-quarters (the bn0 wave starts on the first),
        # panels 1-2 whole; panel 3 in the loop (buffer frees after panel 0).
        wps = {}
        wps[0] = wpool.tile([128, KC * 1024], bf16, tag="wpanel", name="wp0")
        for q in range(4):
            nc.sync.dma_start(
                wps[0][:, q * 2048:(q + 1) * 2048], lw[:, q * 2048:(q + 1) * 2048]
            )
        for p in (1, 2):
            wps[p] = wpool.tile([128, KC * 1024], bf16, tag="wpanel", name=f"wp{p}")
            nc.sync.dma_start(wps[p][:], lw[:, p * 8192:(p + 1) * 8192])

        # leaf predictions, column-packed (bn-major) to keep every
        # activation write at partition base 0
        lpp = keep.tile([8, NBH * I * BN], f32, name="lpp")
        intp_sb = keep.tile([16, BC], f32, name="intp_sb")
        rootp_sb = keep.tile([1, BC], f32, name="rootp_sb")

        inth = {}  # (panel, bn) -> [128, BN] tile: int nodes 4p..4p+3 h^T

        def leaf_mm(wp, bn, gl, k, pg):
            mm(
                pg[:],
                wp[:, k * 1024 + gl * 128:k * 1024 + (gl + 1) * 128],
                xt_sb[:, bn * 4096 + k * BN:bn * 4096 + (k + 1) * BN],
                start=(k == 0),
                stop=(k == KC - 1),
            )

        def leaf_tanh(p, bn, gl, pg):
            lh = work.tile([128, BN], bf16, tag="lh", name=f"lh{p}{bn}{gl}")
            nc.scalar.activation(
                lh[:], pg[:], Tanh, bias=lb_sb[:, GPP * p + gl:GPP * p + gl + 1]
            )
            return lh

        def emit_comb(p, bn, il, ith, lh0, lh1):
            """Fused internal-trans + leaf-predict matmul pair + posts.

            pcomb rows 0:32 accumulate node (4p+il)'s hidden pre-activation
            over its two child groups; rows 32:40 pick up the 8 leaf predict
            dots via the block-diagonal columns (zeros elsewhere)."""
            i = 4 * p + il
            pcomb = psum.tile([64, BN], f32, tag="pcomb", bufs=3,
                              name=f"pc{p}{bn}{il}")
            for j, lh in ((0, lh0), (1, lh1)):
                mm(
                    pcomb[:],
                    cw_sb[:, (2 * i + j) * 64:(2 * i + j + 1) * 64],
                    lh[:],
                    start=(j == 0),
                    stop=(j == 1),
                    skip_group_check=True,
                )
            nc.scalar.activation(
                ith[32 * il:32 * il + 32, :],
                pcomb[0:32, :],
                Tanh,
                bias=intb_sb[32 * il:32 * il + 32, p:p + 1],
            )
            nc.scalar.activation(
                lpp[0:8, (bn * I + i) * BN:(bn * I + i + 1) * BN],
                pcomb[32:40, :],
                Tanh,
                bias=lbp8_sb[:, i:i + 1],
            )
            return pcomb

        def flush_outl(p, bn, eng):
            lo = (bn * I + 4 * p) * BN
            hi = (bn * I + 4 * (p + 1)) * BN
            eng.dma_start(outl[0:8, lo:hi], lpp[0:8, lo:hi])

        pending = []  # deferred (comb il3 + flush) from the previous (p, bn)

        # --- leaf + internal levels ----------------------------------------
        for p in range(NPANEL):
            if p in wps:
                wp = wps[p]
            else:
                wp = wpool.tile([128, KC * 1024], bf16, tag="wpanel", name=f"wp{p}")
                nc.sync.dma_start(wp[:], lw[:, p * 8192:(p + 1) * 8192])
            for bn in range(NBH):
                ith = keep.tile([128, BN], bf16, tag=f"inth{p}{bn}",
                                name=f"inth{p}{bn}")
                lhs = {}

                if p == 0 and bn == 0:
                    # k-outer waves (5 then 3 groups): matmuls start as soon
                    # as the first xt/wp chunks land, and each arriving chunk
                    # feeds a full wave of matmuls. Combs interleave between
                    # waves with at least a tanh of slack.
                    pgs = {}
                    for g0, cnt in ((0, 5), (5, 3)):
                        for q in range(cnt):
                            pgs[g0 + q] = psum.tile(
                                [128, BN], f32, tag="pg", bufs=5,
                                name=f"pgko{g0 + q}")
                        for k in range(KC):
                            for q in range(cnt):
                                leaf_mm(wp, bn, g0 + q, k, pgs[g0 + q])
                        if g0 == 0:
                            for g in (0, 1, 2, 3):
                                lhs[g] = leaf_tanh(p, bn, g, pgs[g])
                            for il in (0, 1):
                                emit_comb(p, bn, il, ith,
                                          lhs[2 * il], lhs[2 * il + 1])
                            lhs[4] = leaf_tanh(p, bn, 4, pgs[4])
                        else:
                            for g in (5, 6):
                                lhs[g] = leaf_tanh(p, bn, g, pgs[g])
                            emit_comb(p, bn, 2, ith, lhs[4], lhs[5])
                            lhs[7] = leaf_tanh(p, bn, 7, pgs[7])
                else:
                    for g in range(GPP):
                        pg = psum.tile([128, BN], f32, tag="pg", bufs=5,
                                       name=f"pg{p}{bn}{g}")
                        for k in range(KC):
                            leaf_mm(wp, bn, g, k, pg)
                        lhs[g] = leaf_tanh(p, bn, g, pg)
                        if g == 0:
                            while pending:
                                pending.pop(0)()
                        if g >= 2 and g % 2 == 0:
                            il = (g - 2) // 2
                            emit_comb(p, bn, il, ith,
                                      lhs[2 * il], lhs[2 * il + 1])

                inth[(p, bn)] = ith

                if p < NPANEL - 1:
                    # defer this half's last comb + predict flush until the
                    # next (p, bn)'s first group is streaming, so the PE
                    # never waits on tanh g7
                    def deferred(p=p, bn=bn, ith=ith, lh6=lhs[6], lh7=lhs[7]):
                        emit_comb(p, bn, 3, ith, lh6, lh7)
                        flush_outl(p, bn, nc.gpsimd)
                    pending.append(deferred)
                else:
                    # final panel: comb il3 now, then the fused int-predict +
                    # root chain. Root matmuls q=0..2 stream while the scalar
                    # engine finishes ith rows 96:128 for q=3.
                    emit_comb(p, bn, 3, ith, lhs[6], lhs[7])
                    prc = psum.tile([48, BN], f32, tag="pcomb", bufs=3,
                                    name=f"prc{bn}")
                    for q in range(NPANEL - 1):
                        mm(prc[:], rw2_sb[:, 48 * q:48 * (q + 1)],
                           inth[(q, bn)][:], start=(q == 0), stop=False,
                           skip_group_check=True)
                    mm(prc[:], rw2_sb[:, 144:192], ith[:], start=False,
                       stop=True, skip_group_check=True)
                    eng = nc.scalar if bn == NBH - 1 else nc.gpsimd
                    rh = work.tile([32, BN], bf16, tag="rh", name=f"rh{bn}")
                    nc.scalar.activation(rh[:], prc[0:32, :], Tanh,
                                         bias=rootb_sb[:, 0:1])
                    nc.scalar.activation(
                        intp_sb[:, bn * BN:bn * BN + BN], prc[32:48, :], Tanh,
                        bias=intbp_sb[:, 0:1],
                    )
                    eng.dma_start(
                        outi[0:16, bn * BN:bn * BN + BN],
                        intp_sb[:, bn * BN:bn * BN + BN],
                    )
                    prp = psum.tile([1, BN], f32, tag="pcomb", bufs=3,
                                    name=f"prp{bn}")
                    mm(prp[:], rootwp_sb[:], rh[:], start=True, stop=True)
                    nc.scalar.activation(
                        rootp_sb[0:1, bn * BN:bn * BN + BN], prp[:], Tanh,
                        bias=rootbp_sb[:, 0:1],
                    )
                    eng.dma_start(
                        outi[16:17, bn * BN:bn * BN + BN],
                        rootp_sb[0:1, bn * BN:bn * BN + BN],
                    )
                    flush_outl(p, bn, eng)

    nc.compile()
    return nc


def _pack_weights(inp):
    import ml_dtypes

    f = np.float32
    bf = ml_dtypes.bfloat16
    leaf_b = np.asarray(inp["leaf_b"], f)
    int_W = np.asarray(inp["int_W"], f)
    int_b = np.asarray(inp["int_b"], f)
    root_W = np.asarray(inp["root_W"], f)
    root_b = np.asarray(inp["root_b"], f)
    leaf_Wp = np.asarray(inp["leaf_Wp"], f)
    leaf_bp = np.asarray(inp["leaf_bp"], f)
    int_Wp = np.asarray(inp["int_Wp"], f)
    int_bp = np.asarray(inp["int_bp"], f)
    root_Wp = np.asarray(inp["root_Wp"], f)
    root_bp = np.asarray(inp["root_bp"], f)

    w = {}
    # lwt[d, l*32+h] = leaf_W[l, d, h]; repack to
    # [p, panel*8192 + k*1024 + q] = lwt[k*128+p, panel*1024+q]
    lwt = np.asarray(inp["leaf_W"], f).transpose(1, 0, 2).reshape(D, L * H)
    w["lw"] = np.ascontiguousarray(
        lwt.reshape(KC, 128, NPANEL, 1024).transpose(1, 2, 0, 3).reshape(
            128, NPANEL * KC * 1024)
    ).astype(bf)

    cwall = np.zeros((128, 2241), f)
    for i in range(I):
        for j in range(2):
            base = (2 * i + j) * 64
            # int_W chunk j of node i: rows (c*32+h) = child (4j+c) hidden h
            cwall[:, base:base + 32] = int_W[i, 128 * j:128 * (j + 1), :]
            for c in range(4):
                lv = 8 * i + 4 * j + c
                cwall[c * 32:(c + 1) * 32, base + 32 + 4 * j + c] = leaf_Wp[lv, :, 0]
    for q in range(NPANEL):
        cwall[:, 2048 + 48 * q:2048 + 48 * q + 32] = root_W[128 * q:128 * (q + 1), :]
        for c in range(4):
            iv = 4 * q + c
            cwall[c * 32:(c + 1) * 32, 2048 + 48 * q + 32 + 4 * q + c] = (
                int_Wp[iv, :, 0])
    cwall[0:32, 2240] = root_Wp[:, 0]
    w["cwall"] = cwall.astype(bf)

    biasall = np.zeros((128, 55), f)
    biasall[:, 0:32] = leaf_b.reshape(32, 128).T
    biasall[:, 32:36] = int_b.reshape(4, 128).T
    biasall[0:8, 36:52] = leaf_bp.reshape(16, 8).T
    biasall[0:16, 52] = int_bp[:, 0]
    biasall[0:32, 53] = root_b
    biasall[0, 54] = root_bp[0]
    w["biasall"] = biasall
    return w


def kernel(**inputs):
    import ml_dtypes

    from concourse.bass_utils import run_bass_kernel_spmd

    nc = _CACHE.get("nc")
    if nc is None:
        nc = _CACHE["nc"] = _build_nc()

    x = np.asarray(inputs["x"], np.float32)
    w = _pack_weights(inputs)
    in_maps = []
    for c in range(NCORES):
        m = dict(w)
        # [p, bn*4096 + k*512 + cc] = x[c*BC + bn*512 + cc, k*128 + p]
        xc = x[c * BC:(c + 1) * BC, :].reshape(NBH, BN, KC, 128)
        m["xt"] = np.ascontiguousarray(
            xc.transpose(3, 0, 2, 1).reshape(128, NBH * KC * BN)
        ).astype(ml_dtypes.bfloat16)
        in_maps.append(m)

    res = run_bass_kernel_spmd(nc, in_maps, core_ids=list(range(NCORES)))
    _CACHE["last_res"] = res
    outs = []
    for c in range(NCORES):
        # outl[r, (bn*16 + i)*512 + cc] -> leaf 8i+r at batch bn*512+cc
        ol = res.results[c]["outl"].reshape(8, NBH, I, BN)
        leafp = ol.transpose(2, 0, 1, 3).reshape(L, BC)
        outs.append(np.concatenate([leafp, res.results[c]["outi"]], axis=0))
    full = np.concatenate([o[:, :, None] for o in outs], axis=1)  # [145, B, 1]
    return full.astype(np.float32)
